# revision 22
# baseline (speedup 1.0000x reference)
"""Trainium2 Bass kernel for causal MultiHeadAttention (B=2, S=2048, E=1024, H=16).

Sharding: 8 cores = 2 (batch) x 4 (head groups of 4, Megatron-style).
Each core computes, for its batch b and head group g:
  - Q/K projections into transposed layout qhT/khT [256, S]  (256 = 4 heads x 64)
  - V projection into natural layout vh [S, 256] with a ones-column per head
  - causal attention with scores kept transposed [k, q]; softmax denominators
    come out of the PV matmul via the ones-column; no max-subtraction needed
    (|scores/sqrt(D)| <~ 6 so exp is well within fp32 range; masked entries are
    zeroed AFTER exp, which matches the reference's -1e9 masking exactly)
  - partial output projection attn_concat @ Wo[rows of g]  -> [S, E]
Host sums the 4 partials per batch and adds bo.

All matmul operands are float16 (full PE rate, fp32 PSUM accumulation;
fp32/fp32r operands trigger PE power-throttling on TRN2 and run ~2x slower).
Emission interleaves projection/output-projection matmuls into the attention
stream as PE filler so the PE clock stays un-throttled (HAM K=8/8), softmax
denominators ride a ones-column in the PV matmul, and 1/sum is computed as
exp(-ln(sum)) on the Scalar engine to keep the slow multi-pass reciprocal off
the Vector engine's FIFO.

Schedule notes:
  - all DRAM inputs are pre-transposed on the host into per-partition-
    contiguous layouts ([P, QC, KT, NQ] activations, [P, KT, CW] weights) so
    every DMA moves 4-8KB contiguous runs per partition at full HBM rate with
    cheap descriptor generation
  - prologue DMA stream is explicitly ordered K -> Q -> V+masks -> chunk-1 x
    (sync-queue issue order == transfer priority); tiny constant loads issue
    from the idle Scalar/GpSimd queues
  - attention chunk 0 is interleaved into the V projection per k-block so the
    Scalar engine's exp stream starts ~8us earlier (rounds 2-3 are exp-bound)
  - vh is padded to 128 weight columns (64 data + ones + 63 zeros) so the PV
    LDWEIGHTS gets Fast-Weight-Load and pipelines behind in-flight matmuls
  - softmax epilogue: 2 LNs into one [1,2,NQ] tile + a single fp16 EXP; the
    broadcast runs on GpSimd mid-kernel but on the PE (K=1 matmul per head)
    for the last head-pair, where GpSimd's op+drain latency would otherwise
    sit on the critical tail
  - round 3 (exp-bound) absorbs the wo matmuls of chunks 0-1 as PE filler;
    all of wo(2) is held back to keep the PE busy (and the HAM clock warm)
    through the final epilogue chain; final output casts alternate
    Scalar/Vector and final output DMAs alternate Sync/GpSimd issue queues
"""

import numpy as np

B, S, E, H = 2, 2048, 1024, 16
D = E // H            # 64 head dim
HL = 4                # heads per core
CW = HL * D           # 256 local channels
P = 128
NQ = 512              # q-chunk (one fp32 PSUM bank)
KT = E // P           # 8 contraction tiles for the input projections
D1 = D + 1            # head slot in vh (+ ones column)

_CACHE = {}


def _pin_act_table(mybir, bacc):
    """Force all activations onto one LUT set containing exp+ln+identity, so
    the ACT engine never reloads tables mid-kernel (1.3us per reload)."""
    from concourse.hw_specs import get_activation_tables

    need = {
        mybir.ActivationFunctionType.Exp,
        mybir.ActivationFunctionType.Ln,
        mybir.ActivationFunctionType.Identity,
    }
    orig = get_activation_tables("gen3")
    target = next(n for n, fs in orig.items() if need <= fs)
    pinned = {n: (fs if n == target else set()) for n, fs in orig.items()}
    bacc.get_activation_tables = lambda arch: pinned


def _build(nc_s=S, num_devices=8):
    import concourse.mybir as mybir
    import concourse.tile as tile
    from concourse import bacc

    _pin_act_table(mybir, bacc)

    f32 = mybir.dt.float32
    h16 = mybir.dt.float16
    Identity = mybir.ActivationFunctionType.Identity
    Ln = mybir.ActivationFunctionType.Ln
    Exp = mybir.ActivationFunctionType.Exp

    QC = nc_s // NQ        # q-chunks
    SB = nc_s // P         # S blocks of 128

    nc = bacc.Bacc(
        "TRN2", target_bir_lowering=False, debug=False, num_devices=num_devices
    )

    def din(name, shape, dt=f32):
        return nc.dram_tensor(name, list(shape), dt, kind="ExternalInput").ap()

    xqt = din("xqt", (P, QC, KT, NQ), h16)
    xkt = din("xkt", (P, QC, KT, NQ), h16)
    xvt = din("xvt", (P, QC, KT, NQ), h16)
    wq = din("wq", (P, KT, CW), h16)
    wk = din("wk", (P, KT, CW), h16)
    wv = din("wv", (P, KT, CW), h16)
    wo = din("wo", (P, CW // P, E), h16)
    bq = din("bq", (P, 2))
    bk = din("bk", (P, 2))
    bv = din("bv", (1, CW), h16)
    masks = din("masks", (P, 4 * NQ), h16)
    onesd = din("ones", (P, P), h16)
    out = nc.dram_tensor("out", [nc_s, E], h16, kind="ExternalOutput").ap()

    with tile.TileContext(nc) as tc:
        with (
            tc.tile_pool(name="singles", bufs=1) as singles,
            tc.tile_pool(name="xpool", bufs=6) as xpool,
            tc.tile_pool(name="exp", bufs=10) as exp_pool,
            tc.tile_pool(name="outp", bufs=4) as out_pool,
            tc.tile_pool(name="small", bufs=6) as small_pool,
            tc.tile_pool(name="stage", bufs=6) as stage_pool,
            tc.tile_pool(name="proj_ps", bufs=2, space="PSUM") as proj_ps,
            tc.tile_pool(name="scores_ps", bufs=2, space="PSUM") as scores_ps,
            tc.tile_pool(name="attn_ps", bufs=2, space="PSUM") as attn_ps,
        ):
            sy = nc.sync

            # --- persistent SBUF tensors -------------------------------------
            wq_sb = singles.tile([P, KT, CW], h16, tag="wq")
            wk_sb = singles.tile([P, KT, CW], h16, tag="wk")
            wv_sb = singles.tile([P, KT, CW], h16, tag="wv")
            wo_sb = singles.tile([P, CW // P, E], h16, tag="wo")
            masks_sb = singles.tile([P, 4, NQ], h16, tag="masks")
            bq_sb = singles.tile([P, 2], f32, tag="bq")
            bk_sb = singles.tile([P, 2], f32, tag="bk")
            bv_row = singles.tile([1, CW], h16, tag="bv")
            ones_col = singles.tile([1, P], h16, tag="ones")
            ones_sb = singles.tile([P, SB * HL], h16, tag="ones_sb")

            qhT = [singles.tile([P, nc_s], h16, name=f"qhT{m}", tag=f"qhT{m}") for m in range(2)]
            khT = [singles.tile([P, nc_s], h16, name=f"khT{m}", tag=f"khT{m}") for m in range(2)]
            atT = [singles.tile([P, nc_s], h16, name=f"atT{m}", tag=f"atT{m}") for m in range(2)]
            # vh is padded to 128 weight columns (64 data + 1 ones + 63
            # zeros): LDWEIGHTS only gets Fast-Weight-Load + background-
            # buffer pipelining at NumWeights==128, otherwise every PV
            # matmul pays a ~110ns serialized weight load
            vh = singles.tile([P, SB, HL, P], h16, tag="vh")

            # --- prologue DMA thunks (sync issue order == transfer priority;
            # tiny constant loads go out on the idle Vector/GpSimd queues so
            # they don't serialize the critical K/Q/V stream: each
            # DMA_DIRECT2D issue costs ~0.6us on its queue) -----------------
            def t_wk_p1():
                sy.dma_start(out=wk_sb[:, :1, :], in_=wk[:, :1, :])

            def t_wk_p2():
                sy.dma_start(out=wk_sb[:, 1:, :], in_=wk[:, 1:, :])
                nc.scalar.dma_start(out=bk_sb, in_=bk)

            def t_wq():
                sy.dma_start(out=wq_sb, in_=wq)
                nc.scalar.dma_start(out=bq_sb, in_=bq)

            def t_wv():
                sy.dma_start(out=wv_sb, in_=wv)
                nc.scalar.dma_start(out=bv_row, in_=bv)
                nc.scalar.dma_start(out=ones_col, in_=onesd[0:1, :])

            def t_attn_consts():
                sy.dma_start(
                    out=masks_sb, in_=masks.rearrange("p (j n) -> p j n", n=NQ)
                )
                nc.gpsimd.dma_start(out=ones_sb, in_=onesd[:, 0 : SB * HL])
                nc.vector.tensor_copy(
                    out=vh[:, :, :, D:D1],
                    in_=ones_sb.rearrange("p (a b) -> p a b", b=HL).unsqueeze(3),
                )
                nc.vector.memset(vh[:, :, :, D1:], 0.0)

            def t_wo():
                sy.dma_start(out=wo_sb, in_=wo)

            # --- stage helpers (thunk-list builders) -------------------------
            def load_x_thunk(src, c, holder, key, parts=None):
                def t(part=None):
                    if part is None or part == 0:
                        tl = xpool.tile([P, KT, NQ], h16, name="xchunk", tag="xchunk")
                        holder[key] = tl
                    tl = holder[key]
                    rsrc = src[:, c]
                    h = KT // 2
                    if c == 0:
                        pieces = [(0, 1), (1, h), (h, KT)]
                    else:
                        pieces = [(0, h), (h, KT)]
                    sel = pieces if part is None else [pieces[part]]
                    for lo, hi in sel:
                        sy.dma_start(out=tl[:, lo:hi, :], in_=rsrc[:, lo:hi, :])
                if parts is None:
                    return [t]
                return [
                    (lambda p: (lambda: t(p)))(p) for p in range(parts)
                ]

            def proj_qk_thunks(c, holder, key, w_sb, b_sb, dstT):
                thunks = []
                pss = {}
                for m in range(2):
                    def mk_mm(m, kt):
                        def t():
                            if kt == 0:
                                pss[m] = proj_ps.tile([P, NQ], f32, name="proj", tag="proj")
                            nc.tensor.matmul(
                                pss[m],
                                w_sb[:, kt, m * P : (m + 1) * P],
                                holder[key][:, kt, :],
                                start=(kt == 0),
                                stop=(kt == KT - 1),
                            )
                        return t
                    for kt in range(KT):
                        thunks.append(mk_mm(m, kt))
                    def mk_copy(m):
                        def t():
                            nc.vector.tensor_scalar_add(
                                out=dstT[m][:, c * NQ : (c + 1) * NQ],
                                in0=pss[m],
                                scalar1=b_sb[:, m : m + 1],
                            )
                        return t
                    thunks.append(mk_copy(m))
                return thunks

            def proj_v_thunks(c, holder, key):
                thunks = []
                pss = {}
                for mb in range(4):
                    j = 4 * c + mb
                    def mk_mm(mb, kt):
                        def t():
                            if kt == 0:
                                pss[mb] = proj_ps.tile([P, NQ], f32, name="proj", tag="proj")
                            nc.tensor.matmul(
                                pss[mb][:, :CW],
                                holder[key][:, kt, mb * P : (mb + 1) * P],
                                wv_sb[:, kt, :],
                                start=(kt == 0),
                                stop=False,
                            )
                        return t
                    for kt in range(KT):
                        thunks.append(mk_mm(mb, kt))
                    def mk_tail(mb, j):
                        def t():
                            nc.tensor.matmul(
                                pss[mb][:, :CW],
                                ones_col,
                                bv_row,
                                start=False,
                                stop=True,
                            )
                            nc.vector.tensor_copy(
                                out=vh[:, j, :, 0:D],
                                in_=pss[mb][:, :CW].rearrange("p (h d) -> p h d", h=HL),
                            )
                        return t
                    thunks.append(mk_tail(mb, j))
                return thunks

            def attn_thunks(c, split=False):
                """Builds the attention thunks for chunk c.  With split=True
                (chunk 0) the scores+exp+mask and the PV matmuls come back as
                separate lists so the exp stream can run during the V load/
                projection window instead of queuing behind it in the PE FIFO."""
                thunks = []
                tail = {}
                sc_parts, pv_parts, epi_parts = [], [], []
                nblk = 4 * (c + 1)
                scale = float(1.0 / np.sqrt(D))
                last_chunk = c == QC - 1
                for hp in range(2):
                    ats = {}
                    exs = {}
                    def mk_sc(hp, j, exs):
                        def t():
                            jj = j - 4 * c
                            q0 = jj * P if jj > 0 else 0
                            sc2 = scores_ps.tile([P, 2, NQ], f32, name="sc2", tag="sc2")
                            for hh in range(2):
                                po = hh * D
                                nc.tensor.matmul(
                                    sc2[:, hh, q0:],
                                    khT[hp][po : po + D, j * P : (j + 1) * P],
                                    qhT[hp][po : po + D, c * NQ + q0 : (c + 1) * NQ],
                                    start=True,
                                    stop=True,
                                )
                            ex2 = exp_pool.tile([P, 2, NQ], h16, name="ex2", tag="ex2")
                            nc.scalar.activation(
                                out=ex2[:, :, q0:], in_=sc2[:, :, q0:], func=Exp,
                                scale=scale,
                            )
                            if jj >= 0:
                                for hh in range(2):
                                    exh = ex2[:, hh, q0:]
                                    nc.vector.tensor_mul(
                                        exh, exh, masks_sb[:, jj, q0:]
                                    )
                            exs[j] = ex2
                        return t
                    def mk_pv(hp, j, ats, exs):
                        def t():
                            if j == 0:
                                ats[0] = attn_ps.tile([P, NQ], f32, name="attn", tag="attn")
                                ats[1] = attn_ps.tile([P, NQ], f32, name="attn", tag="attn")
                            jj = j - 4 * c
                            q0 = jj * P if jj > 0 else 0
                            ex2 = exs[j]
                            for hh in range(2):
                                nc.tensor.matmul(
                                    ats[hh][:, q0:],
                                    vh[:, j, 2 * hp + hh, :],
                                    ex2[:, hh, q0:],
                                    start=(j == 0),
                                    stop=(j == nblk - 1),
                                )
                        return t
                    scs = [mk_sc(hp, j, exs) for j in range(nblk)]
                    pvs = [mk_pv(hp, j, ats, exs) for j in range(nblk)]
                    if split:
                        sc_parts.append(scs)
                        pv_parts.append(pvs)
                    else:
                        for j in range(nblk):
                            thunks.append(scs[j])
                            thunks.append(pvs[j])

                    # softmax epilogue.  1/denom = exp(-ln(denom)) on Scalar;
                    # the [1,NQ] reciprocal row is broadcast to 64 partitions
                    # either by GpSimd (hidden mid-kernel) or, on the kernel
                    # tail, by a K=1 PE matmul (ones[1,64].T @ rs) -- the PE
                    # idles there and GpSimd's 1us op + 1.2us drain is the
                    # tail's critical path.  rb is fp16 so the norm multiply
                    # runs in the DVE 2x packed mode.
                    atu = {}
                    lns = {}
                    def mk_stage(hh, ats, atu, lns):
                        def t():
                            if hh == 0:
                                lns["ls"] = small_pool.tile(
                                    [1, 2, NQ], f32, name="ls", tag="ls"
                                )
                            nc.scalar.activation(
                                out=lns["ls"][:, hh, :],
                                in_=ats[hh][D : D + 1, :],
                                func=Ln, scale=1.0,
                            )
                            atu[hh] = stage_pool.tile(
                                [D, NQ], h16, name="atu", tag="atu"
                            )
                            nc.vector.tensor_copy(atu[hh], ats[hh][0:D, :])
                        return t
                    thunks.append(mk_stage(0, ats, atu, lns))
                    thunks.append(mk_stage(1, ats, atu, lns))

                    use_pe_bcast = last_chunk and hp == 1

                    if use_pe_bcast:
                        # return the epilogue in pieces: the main schedule
                        # interleaves independent wo matmuls between the EXPs
                        # and the PE broadcasts so the PE queue never stalls
                        # on the ACT chain (stall >3.4us re-throttles HAM and
                        # the whole tail then runs at 1.2 GHz)
                        rbh = {}
                        def mk_exp(lns, rbh):
                            def t():
                                rs = small_pool.tile(
                                    [1, 2, NQ], h16, name="rs", tag="rs"
                                )
                                nc.scalar.activation(
                                    out=rs, in_=lns["ls"], func=Exp, scale=-1.0
                                )
                                rbh["rs"] = rs
                            return t
                        def mk_bcast(hh, rbh):
                            def t():
                                rbh[("ps", hh)] = attn_ps.tile(
                                    [D, NQ], f32, name="attn", tag="attn"
                                )
                                nc.tensor.matmul(
                                    rbh[("ps", hh)],
                                    ones_col[0:1, 0:D],
                                    rbh["rs"][:, hh, :],
                                    start=True,
                                    stop=True,
                                )
                            return t
                        def mk_mul(hp, hh, atu, rbh):
                            def t():
                                po = hh * D
                                nc.vector.tensor_mul(
                                    atT[hp][po : po + D, c * NQ : (c + 1) * NQ],
                                    atu[hh],
                                    rbh[("ps", hh)],
                                )
                            return t
                        # stage thunks (ln + atu copy) were already appended;
                        # move them into the tail dict, ACT-ordered so hh=0's
                        # reciprocal is ready as early as possible:
                        # ln0 -> exp0 -> ln1 -> exp1
                        st = thunks[-2:]
                        del thunks[-2:]
                        tail["act"] = [st[0], st[1], mk_exp(lns, rbh)]
                        tail["bcast"] = [mk_bcast(0, rbh), mk_bcast(1, rbh)]
                        tail["mul"] = [mk_mul(hp, 0, atu, rbh), mk_mul(hp, 1, atu, rbh)]
                    else:
                        def mk_norm(hp, hh, atu, lns):
                            def t():
                                po = hh * D
                                if hh == 0:
                                    lns["rs"] = small_pool.tile(
                                        [1, 2, NQ], h16, name="rs", tag="rs"
                                    )
                                    nc.scalar.activation(
                                        out=lns["rs"], in_=lns["ls"],
                                        func=Exp, scale=-1.0,
                                    )
                                rb = small_pool.tile([D, NQ], h16, name="rb", tag="rb")
                                nc.gpsimd.partition_broadcast(
                                    rb, lns["rs"][:, hh, :]
                                )
                                nc.vector.tensor_mul(
                                    atT[hp][po : po + D, c * NQ : (c + 1) * NQ],
                                    atu[hh],
                                    rb,
                                )
                            return t
                        thunks.append(mk_norm(hp, 0, atu, lns))
                        thunks.append(mk_norm(hp, 1, atu, lns))
                    if split:
                        epi_parts.append(thunks[-4:])
                        del thunks[-4:]
                if split:
                    return sc_parts, pv_parts, epi_parts
                return thunks, tail

            def wo_thunks(c):
                thunks = []
                for mb in range(4):
                    ms = 4 * c + mb
                    for n in range(2):
                        def mk(ms, n):
                            def t():
                                ps = proj_ps.tile([P, NQ], f32, name="proj", tag="proj")
                                for kt in range(CW // P):
                                    nc.tensor.matmul(
                                        ps,
                                        atT[kt][:, ms * P : (ms + 1) * P],
                                        wo_sb[:, kt, n * NQ : (n + 1) * NQ],
                                        start=(kt == 0),
                                        stop=(kt == CW // P - 1),
                                    )
                                ot = out_pool.tile([P, NQ], h16, name="ot", tag="ot")
                                nc.vector.tensor_copy(ot, ps)
                                sy.dma_start(
                                    out=out[
                                        ms * P : (ms + 1) * P, n * NQ : (n + 1) * NQ
                                    ],
                                    in_=ot,
                                )
                            return t
                        thunks.append(mk(ms, n))
                return thunks

            def wo_tail_waves(c):
                """Final-chunk wo, kt-split: kt=0 (pair-0 atT, ready early)
                can issue during pair-1's norm chain; 4 concurrent PSUM
                groups (2 proj + 2 borrowed scores banks).  Output casts
                alternate Scalar/Vector, output DMA issues alternate
                Sync/GpSimd, so the tail drains ~2x faster."""
                units = [(4 * c + mb, n) for mb in range(4) for n in range(2)]
                waves = []
                for wave in (units[:4], units[4:]):
                    pss = {}
                    def mk_kt0(i, ms, n, pss):
                        def t():
                            if i < 2:
                                pss[i] = proj_ps.tile(
                                    [P, NQ], f32, name="proj", tag="proj"
                                )
                            else:
                                ps2 = scores_ps.tile(
                                    [P, 2, NQ], f32, name="sc2", tag="sc2"
                                )
                                pss[i] = ps2[:, 0, :]
                            nc.tensor.matmul(
                                pss[i],
                                atT[0][:, ms * P : (ms + 1) * P],
                                wo_sb[:, 0, n * NQ : (n + 1) * NQ],
                                start=True,
                                stop=False,
                            )
                        return t
                    def mk_kt1(i, ms, n, pss):
                        def t():
                            nc.tensor.matmul(
                                pss[i],
                                atT[1][:, ms * P : (ms + 1) * P],
                                wo_sb[:, 1, n * NQ : (n + 1) * NQ],
                                start=False,
                                stop=True,
                            )
                            ot = out_pool.tile([P, NQ], h16, name="ot", tag="ot")
                            if i % 2 == 0:
                                nc.vector.tensor_copy(ot, pss[i])
                            else:
                                nc.scalar.activation(
                                    out=ot, in_=pss[i], func=Identity, scale=1.0
                                )
                            eng = sy if i % 2 == 0 else nc.gpsimd
                            eng.dma_start(
                                out=out[ms * P : (ms + 1) * P, n * NQ : (n + 1) * NQ],
                                in_=ot,
                            )
                        return t
                    kt0s = [mk_kt0(i, ms, n, pss) for i, (ms, n) in enumerate(wave)]
                    kt1s = [mk_kt1(i, ms, n, pss) for i, (ms, n) in enumerate(wave)]
                    waves.append((kt0s, kt1s))
                return waves

            def emit_interleaved(primary, filler):
                fi = 0
                n = max(len(primary), 1)
                f = len(filler)
                for i, t in enumerate(primary):
                    t()
                    while fi * n < f * (i + 1):
                        filler[fi]()
                        fi += 1
                for t in filler[fi:]:
                    t()

            # --- main schedule ----------------------------------------------
            holder = {}
            # prologue: K -> Q -> V (scores need q+k first; PV can lag), then
            # attention consts, then chunk-1 x so round-0 filler is never
            # DMA-starved.  Emission order == sync-queue issue order ==
            # transfer priority; wk/xk pieces interleave so the first
            # projection matmul starts one small DMA piece into the stream.
            xk0 = load_x_thunk(xkt, 0, holder, ("xk", 0), parts=3)
            sc0, pv0, epi0 = attn_thunks(0, split=True)
            v0 = proj_v_thunks(0, holder, ("xv", 0))
            K0 = proj_qk_thunks(0, holder, ("xk", 0), wk_sb, bk_sb, khT)
            Q0 = proj_qk_thunks(0, holder, ("xq", 0), wq_sb, bq_sb, qhT)
            # chunk-0 attention is pipelined into the projections: head-pair
            # 0's scores+exp run right after the m=0 K/Q projections (the
            # head-pair only needs the m=0 halves), head-pair 1's after m=1,
            # and each PV follows its vh block.  The Scalar engine's exp
            # stream otherwise idles until every projection is done and then
            # becomes the bottleneck in the late rounds.
            v_att = []
            for mb in range(4):
                v_att += v0[9 * mb : 9 * (mb + 1)]
                v_att.append(pv0[0][mb])
            v_att += epi0[0] + list(pv0[1]) + epi0[1]
            prologue = (
                [t_wk_p1, xk0[0], t_wk_p2, xk0[1], xk0[2]]
                + K0[:9]
                + [t_wq]
                + load_x_thunk(xqt, 0, holder, ("xq", 0))
                + [t_wv, t_attn_consts]
                + load_x_thunk(xvt, 0, holder, ("xv", 0))
                + Q0[:9]
                + list(sc0[0])
                + K0[9:] + Q0[9:]
                + list(sc0[1])
                + v_att
                + load_x_thunk(xkt, 1, holder, ("xk", 1))
                + load_x_thunk(xqt, 1, holder, ("xq", 1))
                + load_x_thunk(xvt, 1, holder, ("xv", 1))
            )
            for t in prologue:
                t()
            kv_deferred = {}
            tail = None
            for c in range(QC):
                kv_filler = kv_deferred.pop(c, [])
                filler = []
                holdback = []
                if c == 0:
                    filler += [t_wo]
                if c == 3:
                    # round 3 is ACT-bound (exp over the longest k-ranges), so
                    # it absorbs the wo matmuls of chunks 0 and 1 as filler;
                    # all of wo(2) is held back to keep the PE warm through
                    # the final softmax epilogue chain
                    wo01 = wo_thunks(0) + wo_thunks(1)
                    filler += wo01[:14]
                    holdback = wo01[14:] + wo_thunks(2)
                if c + 1 < QC:
                    if c + 2 < QC:
                        filler += load_x_thunk(xkt, c + 2, holder, ("xk", c + 2))
                        filler += load_x_thunk(xqt, c + 2, holder, ("xq", c + 2))
                        filler += load_x_thunk(xvt, c + 2, holder, ("xv", c + 2))
                    filler += proj_qk_thunks(
                        c + 1, holder, ("xq", c + 1), wq_sb, bq_sb, qhT
                    )
                    filler += proj_qk_thunks(
                        c + 1, holder, ("xk", c + 1), wk_sb, bk_sb, khT
                    )
                    kv_deferred[c + 1] = proj_v_thunks(
                        c + 1, holder, ("xv", c + 1)
                    )
                if c == 0:
                    for t in filler:
                        t()
                    continue
                att, tail = attn_thunks(c)
                seg1, seg2 = att[: 8 * c], att[8 * c :]
                emit_interleaved(seg1, kv_filler)
                # hold back a quarter of the filler to keep PE fed through the
                # end-of-round normalization chains
                cut = (3 * len(filler)) // 4
                emit_interleaved(seg2[:-8], filler[:cut])
                emit_interleaved(seg2[-8:], filler[cut:])
            # chunks whose wo was not emitted as filler above (QC=4 covers
            # chunks 0..2 in rounds 2/3; smaller QC emits them all here)
            done = {0, 1, 2} if QC == 4 else set()
            for cc in range(QC):
                if cc not in done and cc != QC - 1:
                    for t in wo_thunks(cc):
                        t()
                    holdback = []
            # ---- tail: last head-pair epilogue with independent wo work
            # interleaved so the PE queue never waits on the ACT chain ------
            (w1_kt0, w1_kt1), (w2_kt0, w2_kt1) = wo_tail_waves(QC - 1)
            if tail:
                for t in tail["act"]:       # ln0, exp0, ln1, exp1 (ACT)
                    t()
                for t in holdback:          # held-back wo(2) units (PE)
                    t()
                for t in w1_kt0:            # wave-1 kt0 (PE, needs atT[0])
                    t()
                for t in tail["bcast"]:     # PE broadcasts of 1/denom
                    t()
                for t in tail["mul"]:       # norm multiplies (DVE)
                    t()
            else:
                for t in holdback:
                    t()
                for t in w1_kt0:
                    t()
            for t in w1_kt1 + w2_kt0 + w2_kt1:
                t()

    nc.compile()
    return nc


def _get_nc(nc_s=S):
    if nc_s not in _CACHE:
        _CACHE[nc_s] = _build(nc_s)
    return _CACHE[nc_s]


def make_masks():
    m = np.zeros((P, 4, NQ), np.float32)
    ql = np.arange(NQ)[None, :]
    kl = np.arange(P)[:, None]
    for jj in range(4):
        m[:, jj, :] = (ql >= kl + jj * P).astype(np.float32)
    return m.reshape(P, 4 * NQ)


def _x_layout(x):
    """[S, E] activations -> [P, QC, KT, NQ]: per-partition-contiguous chunks
    so every x DMA is 128 x (up to 8KB) contiguous runs at full HBM rate."""
    QC = S // NQ
    xt = np.ascontiguousarray(x.T).astype(np.float16)          # [E, S]
    return np.ascontiguousarray(
        xt.reshape(KT, P, QC, NQ).transpose(1, 2, 0, 3)
    )


def _w_layout(w):
    # [E, CW] -> [P, KT, CW]
    return np.ascontiguousarray(
        w.reshape(KT, P, CW).transpose(1, 0, 2)
    ).astype(np.float16)


def _wo_layout(w):
    # [CW, E] -> [P, CW//P, E]
    return np.ascontiguousarray(
        w.reshape(CW // P, P, E).transpose(1, 0, 2)
    ).astype(np.float16)


def _b_layout(b):
    # [CW] -> [P, 2]
    return np.ascontiguousarray(b.reshape(2, P).T).astype(np.float32)


def make_in_maps(q, k, v, Wq, bq, Wk, bk, Wv, bv, Wo):
    masks = make_masks()
    in_maps = []
    for core in range(8):
        b, g = divmod(core, 4)
        cs = slice(g * CW, (g + 1) * CW)
        in_maps.append(
            {
                "xqt": _x_layout(q[b]),
                "xkt": _x_layout(k[b]),
                "xvt": _x_layout(v[b]),
                "wq": _w_layout(Wq[:, cs]),
                "wk": _w_layout(Wk[:, cs]),
                "wv": _w_layout(Wv[:, cs]),
                "wo": _wo_layout(Wo[cs, :]),
                "bq": _b_layout(bq[cs]),
                "bk": _b_layout(bk[cs]),
                "bv": np.ascontiguousarray(bv[cs].reshape(1, CW)).astype(np.float16),
                "masks": masks.astype(np.float16),
                "ones": np.ones((P, P), np.float16),
            }
        )
    return in_maps


def run(q, k, v, Wq, bq, Wk, bk, Wv, bv, Wo, bo, **run_kwargs):
    """Returns (output, BassKernelResults)."""
    from concourse.bass_utils import run_bass_kernel_spmd

    q, k, v = (np.asarray(x, np.float32) for x in (q, k, v))
    nc = _get_nc()
    in_maps = make_in_maps(
        q, k, v,
        np.asarray(Wq, np.float32), np.asarray(bq, np.float32),
        np.asarray(Wk, np.float32), np.asarray(bk, np.float32),
        np.asarray(Wv, np.float32), np.asarray(bv, np.float32),
        np.asarray(Wo, np.float32),
    )
    res = run_bass_kernel_spmd(nc, in_maps, list(range(8)), **run_kwargs)
    out = np.zeros((B, S, E), np.float32)
    for core in range(8):
        out[core // 4] += res.results[core]["out"].astype(np.float32)
    out += np.asarray(bo, np.float32)[None, None, :]
    return out, res


def kernel(q, k, v, Wq, bq, Wk, bk, Wv, bv, Wo, bo):
    return run(q, k, v, Wq, bq, Wk, bk, Wv, bv, Wo, bo)[0]


# revision 23
# speedup vs baseline: 1.0171x; 1.0171x over previous
"""Trainium2 Bass kernel for causal MultiHeadAttention (B=2, S=2048, E=1024, H=16).

Sharding: 8 cores = 2 (batch) x 4 (head groups of 4, Megatron-style).
Each core computes, for its batch b and head group g:
  - Q/K projections into transposed layout qhT/khT [256, S]  (256 = 4 heads x 64)
  - V projection into natural layout vh [S, 256] with a ones-column per head
  - causal attention with scores kept transposed [k, q]; softmax denominators
    come out of the PV matmul via the ones-column; no max-subtraction needed
    (|scores/sqrt(D)| <~ 6 so exp is well within fp32 range; masked entries are
    zeroed AFTER exp, which matches the reference's -1e9 masking exactly)
  - partial output projection attn_concat @ Wo[rows of g]  -> [S, E]
Host sums the 4 partials per batch and adds bo.

All matmul operands are float16 (full PE rate, fp32 PSUM accumulation;
fp32/fp32r operands trigger PE power-throttling on TRN2 and run ~2x slower).
Emission interleaves projection/output-projection matmuls into the attention
stream as PE filler so the PE clock stays un-throttled (HAM K=8/8), softmax
denominators ride a ones-column in the PV matmul, and 1/sum is computed as
exp(-ln(sum)) on the Scalar engine to keep the slow multi-pass reciprocal off
the Vector engine's FIFO.

Schedule notes:
  - all DRAM inputs are pre-transposed on the host into per-partition-
    contiguous layouts ([P, QC, KT, NQ] activations, [P, KT, CW] weights) so
    every DMA moves 4-8KB contiguous runs per partition at full HBM rate with
    cheap descriptor generation
  - prologue DMA stream is explicitly ordered K -> Q -> V+masks -> chunk-1 x
    (sync-queue issue order == transfer priority); tiny constant loads issue
    from the idle Scalar/GpSimd queues
  - attention chunk 0 is interleaved into the V projection per k-block so the
    Scalar engine's exp stream starts ~8us earlier (rounds 2-3 are exp-bound)
  - vh is padded to 128 weight columns (64 data + ones + 63 zeros) so the PV
    LDWEIGHTS gets Fast-Weight-Load and pipelines behind in-flight matmuls
  - softmax epilogue: 2 LNs into one [1,2,NQ] tile + a single fp16 EXP; the
    broadcast runs on GpSimd mid-kernel but on the PE (K=1 matmul per head)
    for the last head-pair, where GpSimd's op+drain latency would otherwise
    sit on the critical tail
  - round 3 (exp-bound) absorbs the wo matmuls of chunks 0-1 as PE filler;
    all of wo(2) is held back to keep the PE busy (and the HAM clock warm)
    through the final epilogue chain; final output casts alternate
    Scalar/Vector and final output DMAs alternate Sync/GpSimd issue queues
"""

import numpy as np

B, S, E, H = 2, 2048, 1024, 16
D = E // H            # 64 head dim
HL = 4                # heads per core
CW = HL * D           # 256 local channels
P = 128
NQ = 512              # q-chunk (one fp32 PSUM bank)
KT = E // P           # 8 contraction tiles for the input projections
D1 = D + 1            # head slot in vh (+ ones column)

_CACHE = {}


def _pin_act_table(mybir, bacc):
    """Force all activations onto one LUT set containing exp+ln+identity, so
    the ACT engine never reloads tables mid-kernel (1.3us per reload)."""
    from concourse.hw_specs import get_activation_tables

    need = {
        mybir.ActivationFunctionType.Exp,
        mybir.ActivationFunctionType.Ln,
        mybir.ActivationFunctionType.Identity,
    }
    orig = get_activation_tables("gen3")
    target = next(n for n, fs in orig.items() if need <= fs)
    pinned = {n: (fs if n == target else set()) for n, fs in orig.items()}
    bacc.get_activation_tables = lambda arch: pinned


def _build(nc_s=S, num_devices=8):
    import concourse.mybir as mybir
    import concourse.tile as tile
    from concourse import bacc

    _pin_act_table(mybir, bacc)

    f32 = mybir.dt.float32
    h16 = mybir.dt.float16
    Identity = mybir.ActivationFunctionType.Identity
    Ln = mybir.ActivationFunctionType.Ln
    Exp = mybir.ActivationFunctionType.Exp

    QC = nc_s // NQ        # q-chunks
    SB = nc_s // P         # S blocks of 128

    nc = bacc.Bacc(
        "TRN2", target_bir_lowering=False, debug=False, num_devices=num_devices
    )

    def din(name, shape, dt=f32):
        return nc.dram_tensor(name, list(shape), dt, kind="ExternalInput").ap()

    xqt = din("xqt", (P, QC, KT, NQ), h16)
    xkt = din("xkt", (P, QC, KT, NQ), h16)
    xvt = din("xvt", (P, QC, KT, NQ), h16)
    wq = din("wq", (P, KT, CW), h16)
    wk = din("wk", (P, KT, CW), h16)
    wv = din("wv", (P, KT, CW), h16)
    wo = din("wo", (P, CW // P, E), h16)
    bq = din("bq", (P, 2))
    bk = din("bk", (P, 2))
    bv = din("bv", (1, CW), h16)
    masks = din("masks", (P, 4 * NQ), h16)
    onesd = din("ones", (P, P), h16)
    out = nc.dram_tensor("out", [nc_s, E], h16, kind="ExternalOutput").ap()

    with tile.TileContext(nc) as tc:
        with (
            tc.tile_pool(name="singles", bufs=1) as singles,
            tc.tile_pool(name="xpool", bufs=6) as xpool,
            tc.tile_pool(name="exp", bufs=10) as exp_pool,
            tc.tile_pool(name="outp", bufs=4) as out_pool,
            tc.tile_pool(name="small", bufs=6) as small_pool,
            tc.tile_pool(name="stage", bufs=6) as stage_pool,
            tc.tile_pool(name="proj_ps", bufs=2, space="PSUM") as proj_ps,
            tc.tile_pool(name="scores_ps", bufs=2, space="PSUM") as scores_ps,
            tc.tile_pool(name="attn_ps", bufs=2, space="PSUM") as attn_ps,
        ):
            sy = nc.sync

            # --- persistent SBUF tensors -------------------------------------
            wq_sb = singles.tile([P, KT, CW], h16, tag="wq")
            wk_sb = singles.tile([P, KT, CW], h16, tag="wk")
            wv_sb = singles.tile([P, KT, CW], h16, tag="wv")
            wo_sb = singles.tile([P, CW // P, E], h16, tag="wo")
            masks_sb = singles.tile([P, 4, NQ], h16, tag="masks")
            bq_sb = singles.tile([P, 2], f32, tag="bq")
            bk_sb = singles.tile([P, 2], f32, tag="bk")
            bv_row = singles.tile([1, CW], h16, tag="bv")
            ones_col = singles.tile([1, P], h16, tag="ones")
            ones_sb = singles.tile([P, SB * HL], h16, tag="ones_sb")

            qhT = [singles.tile([P, nc_s], h16, name=f"qhT{m}", tag=f"qhT{m}") for m in range(2)]
            khT = [singles.tile([P, nc_s], h16, name=f"khT{m}", tag=f"khT{m}") for m in range(2)]
            atT = [singles.tile([P, nc_s], h16, name=f"atT{m}", tag=f"atT{m}") for m in range(2)]
            # vh is padded to 128 weight columns (64 data + 1 ones + 63
            # zeros): LDWEIGHTS only gets Fast-Weight-Load + background-
            # buffer pipelining at NumWeights==128, otherwise every PV
            # matmul pays a ~110ns serialized weight load
            vh = singles.tile([P, SB, HL, P], h16, tag="vh")

            # --- prologue DMA thunks (sync issue order == transfer priority;
            # tiny constant loads go out on the idle Vector/GpSimd queues so
            # they don't serialize the critical K/Q/V stream: each
            # DMA_DIRECT2D issue costs ~0.6us on its queue) -----------------
            def t_wk_p1():
                sy.dma_start(out=wk_sb[:, :1, :], in_=wk[:, :1, :])

            def t_wk_p2():
                sy.dma_start(out=wk_sb[:, 1:, :], in_=wk[:, 1:, :])
                nc.scalar.dma_start(out=bk_sb, in_=bk)

            def t_wq():
                sy.dma_start(out=wq_sb, in_=wq)
                nc.scalar.dma_start(out=bq_sb, in_=bq)

            def t_wv():
                sy.dma_start(out=wv_sb, in_=wv)
                nc.scalar.dma_start(out=bv_row, in_=bv)
                nc.scalar.dma_start(out=ones_col, in_=onesd[0:1, :])

            def t_attn_consts():
                sy.dma_start(
                    out=masks_sb, in_=masks.rearrange("p (j n) -> p j n", n=NQ)
                )
                nc.gpsimd.dma_start(out=ones_sb, in_=onesd[:, 0 : SB * HL])
                nc.vector.tensor_copy(
                    out=vh[:, :, :, D:D1],
                    in_=ones_sb.rearrange("p (a b) -> p a b", b=HL).unsqueeze(3),
                )
                nc.vector.memset(vh[:, :, :, D1:], 0.0)

            def t_wo():
                sy.dma_start(out=wo_sb, in_=wo)

            # --- stage helpers (thunk-list builders) -------------------------
            def load_x_thunk(src, c, holder, key, parts=None):
                def t(part=None):
                    if part is None or part == 0:
                        tl = xpool.tile([P, KT, NQ], h16, name="xchunk", tag="xchunk")
                        holder[key] = tl
                    tl = holder[key]
                    rsrc = src[:, c]
                    h = KT // 2
                    if c == 0:
                        pieces = [(0, 1), (1, h), (h, KT)]
                    else:
                        pieces = [(0, h), (h, KT)]
                    sel = pieces if part is None else [pieces[part]]
                    for lo, hi in sel:
                        sy.dma_start(out=tl[:, lo:hi, :], in_=rsrc[:, lo:hi, :])
                if parts is None:
                    return [t]
                return [
                    (lambda p: (lambda: t(p)))(p) for p in range(parts)
                ]

            def proj_qk_thunks(c, holder, key, w_sb, b_sb, dstT):
                thunks = []
                pss = {}
                for m in range(2):
                    def mk_mm(m, kt):
                        def t():
                            if kt == 0:
                                pss[m] = proj_ps.tile([P, NQ], f32, name="proj", tag="proj")
                            nc.tensor.matmul(
                                pss[m],
                                w_sb[:, kt, m * P : (m + 1) * P],
                                holder[key][:, kt, :],
                                start=(kt == 0),
                                stop=(kt == KT - 1),
                            )
                        return t
                    for kt in range(KT):
                        thunks.append(mk_mm(m, kt))
                    def mk_copy(m):
                        def t():
                            nc.vector.tensor_scalar_add(
                                out=dstT[m][:, c * NQ : (c + 1) * NQ],
                                in0=pss[m],
                                scalar1=b_sb[:, m : m + 1],
                            )
                        return t
                    thunks.append(mk_copy(m))
                return thunks

            def proj_v_thunks(c, holder, key):
                thunks = []
                pss = {}
                for mb in range(4):
                    j = 4 * c + mb
                    def mk_mm(mb, kt):
                        def t():
                            if kt == 0:
                                pss[mb] = proj_ps.tile([P, NQ], f32, name="proj", tag="proj")
                            nc.tensor.matmul(
                                pss[mb][:, :CW],
                                holder[key][:, kt, mb * P : (mb + 1) * P],
                                wv_sb[:, kt, :],
                                start=(kt == 0),
                                stop=False,
                            )
                        return t
                    for kt in range(KT):
                        thunks.append(mk_mm(mb, kt))
                    def mk_tail(mb, j):
                        def t():
                            nc.tensor.matmul(
                                pss[mb][:, :CW],
                                ones_col,
                                bv_row,
                                start=False,
                                stop=True,
                            )
                            nc.vector.tensor_copy(
                                out=vh[:, j, :, 0:D],
                                in_=pss[mb][:, :CW].rearrange("p (h d) -> p h d", h=HL),
                            )
                        return t
                    thunks.append(mk_tail(mb, j))
                return thunks

            def attn_thunks(c, split=False):
                """Builds the attention thunks for chunk c.  With split=True
                (chunk 0) the scores+exp+mask and the PV matmuls come back as
                separate lists so the exp stream can run during the V load/
                projection window instead of queuing behind it in the PE FIFO."""
                thunks = []
                tail = {}
                sc_parts, pv_parts, epi_parts = [], [], []
                nblk = 4 * (c + 1)
                scale = float(1.0 / np.sqrt(D))
                last_chunk = c == QC - 1
                for hp in range(2):
                    ats = {}
                    exs = {}
                    def mk_sc(hp, j, exs):
                        def t():
                            jj = j - 4 * c
                            q0 = jj * P if jj > 0 else 0
                            sc2 = scores_ps.tile([P, 2, NQ], f32, name="sc2", tag="sc2")
                            for hh in range(2):
                                po = hh * D
                                nc.tensor.matmul(
                                    sc2[:, hh, q0:],
                                    khT[hp][po : po + D, j * P : (j + 1) * P],
                                    qhT[hp][po : po + D, c * NQ + q0 : (c + 1) * NQ],
                                    start=True,
                                    stop=True,
                                )
                            ex2 = exp_pool.tile([P, 2, NQ], h16, name="ex2", tag="ex2")
                            nc.scalar.activation(
                                out=ex2[:, :, q0:], in_=sc2[:, :, q0:], func=Exp,
                                scale=scale,
                            )
                            if jj >= 0:
                                for hh in range(2):
                                    exh = ex2[:, hh, q0:]
                                    nc.vector.tensor_mul(
                                        exh, exh, masks_sb[:, jj, q0:]
                                    )
                            exs[j] = ex2
                        return t
                    def mk_pv(hp, j, ats, exs):
                        def t():
                            if j == 0:
                                ats[0] = attn_ps.tile([P, NQ], f32, name="attn", tag="attn")
                                ats[1] = attn_ps.tile([P, NQ], f32, name="attn", tag="attn")
                            jj = j - 4 * c
                            q0 = jj * P if jj > 0 else 0
                            ex2 = exs[j]
                            for hh in range(2):
                                nc.tensor.matmul(
                                    ats[hh][:, q0:],
                                    vh[:, j, 2 * hp + hh, :],
                                    ex2[:, hh, q0:],
                                    start=(j == 0),
                                    stop=(j == nblk - 1),
                                )
                        return t
                    scs = [mk_sc(hp, j, exs) for j in range(nblk)]
                    pvs = [mk_pv(hp, j, ats, exs) for j in range(nblk)]
                    if split:
                        sc_parts.append(scs)
                        pv_parts.append(pvs)
                    else:
                        for j in range(nblk):
                            thunks.append(scs[j])
                            thunks.append(pvs[j])

                    # softmax epilogue.  1/denom = exp(-ln(denom)) on Scalar;
                    # the [1,NQ] reciprocal row is broadcast to 64 partitions
                    # either by GpSimd (hidden mid-kernel) or, on the kernel
                    # tail, by a K=1 PE matmul (ones[1,64].T @ rs) -- the PE
                    # idles there and GpSimd's 1us op + 1.2us drain is the
                    # tail's critical path.  rb is fp16 so the norm multiply
                    # runs in the DVE 2x packed mode.
                    atu = {}
                    lns = {}
                    def mk_stage(hh, ats, atu, lns):
                        def t():
                            if hh == 0:
                                lns["ls"] = small_pool.tile(
                                    [1, 2, NQ], f32, name="ls", tag="ls"
                                )
                            nc.scalar.activation(
                                out=lns["ls"][:, hh, :],
                                in_=ats[hh][D : D + 1, :],
                                func=Ln, scale=1.0,
                            )
                            atu[hh] = stage_pool.tile(
                                [D, NQ], h16, name="atu", tag="atu"
                            )
                            nc.vector.tensor_copy(atu[hh], ats[hh][0:D, :])
                        return t
                    thunks.append(mk_stage(0, ats, atu, lns))
                    thunks.append(mk_stage(1, ats, atu, lns))

                    use_pe_bcast = last_chunk and hp == 1

                    if use_pe_bcast:
                        # return the epilogue in pieces: the main schedule
                        # interleaves independent wo matmuls between the EXPs
                        # and the PE broadcasts so the PE queue never stalls
                        # on the ACT chain (stall >3.4us re-throttles HAM and
                        # the whole tail then runs at 1.2 GHz)
                        rbh = {}
                        def mk_exp(lns, rbh):
                            def t():
                                rs = small_pool.tile(
                                    [1, 2, NQ], h16, name="rs", tag="rs"
                                )
                                nc.scalar.activation(
                                    out=rs, in_=lns["ls"], func=Exp, scale=-1.0
                                )
                                rbh["rs"] = rs
                            return t
                        def mk_bcast(hh, rbh):
                            def t():
                                rbh[("ps", hh)] = attn_ps.tile(
                                    [D, NQ], f32, name="attn", tag="attn"
                                )
                                nc.tensor.matmul(
                                    rbh[("ps", hh)],
                                    ones_col[0:1, 0:D],
                                    rbh["rs"][:, hh, :],
                                    start=True,
                                    stop=True,
                                )
                            return t
                        def mk_mul(hp, hh, atu, rbh):
                            def t():
                                po = hh * D
                                nc.vector.tensor_mul(
                                    atT[hp][po : po + D, c * NQ : (c + 1) * NQ],
                                    atu[hh],
                                    rbh[("ps", hh)],
                                )
                            return t
                        # stage thunks (ln + atu copy) were already appended;
                        # move them into the tail dict, ACT-ordered so hh=0's
                        # reciprocal is ready as early as possible:
                        # ln0 -> exp0 -> ln1 -> exp1
                        st = thunks[-2:]
                        del thunks[-2:]
                        tail["act"] = [st[0], st[1], mk_exp(lns, rbh)]
                        tail["bcast"] = [mk_bcast(0, rbh), mk_bcast(1, rbh)]
                        tail["mul"] = [mk_mul(hp, 0, atu, rbh), mk_mul(hp, 1, atu, rbh)]
                    else:
                        def mk_norm(hp, hh, atu, lns):
                            def t():
                                po = hh * D
                                if hh == 0:
                                    lns["rs"] = small_pool.tile(
                                        [1, 2, NQ], h16, name="rs", tag="rs"
                                    )
                                    nc.scalar.activation(
                                        out=lns["rs"], in_=lns["ls"],
                                        func=Exp, scale=-1.0,
                                    )
                                rb = small_pool.tile([D, NQ], h16, name="rb", tag="rb")
                                nc.gpsimd.partition_broadcast(
                                    rb, lns["rs"][:, hh, :]
                                )
                                nc.vector.tensor_mul(
                                    atT[hp][po : po + D, c * NQ : (c + 1) * NQ],
                                    atu[hh],
                                    rb,
                                )
                            return t
                        thunks.append(mk_norm(hp, 0, atu, lns))
                        thunks.append(mk_norm(hp, 1, atu, lns))
                    if split:
                        epi_parts.append(thunks[-4:])
                        del thunks[-4:]
                if split:
                    return sc_parts, pv_parts, epi_parts
                return thunks, tail

            def wo_thunks(c):
                thunks = []
                for mb in range(4):
                    ms = 4 * c + mb
                    for n in range(2):
                        def mk(ms, n):
                            def t():
                                ps = proj_ps.tile([P, NQ], f32, name="proj", tag="proj")
                                for kt in range(CW // P):
                                    nc.tensor.matmul(
                                        ps,
                                        atT[kt][:, ms * P : (ms + 1) * P],
                                        wo_sb[:, kt, n * NQ : (n + 1) * NQ],
                                        start=(kt == 0),
                                        stop=(kt == CW // P - 1),
                                    )
                                ot = out_pool.tile([P, NQ], h16, name="ot", tag="ot")
                                nc.vector.tensor_copy(ot, ps)
                                sy.dma_start(
                                    out=out[
                                        ms * P : (ms + 1) * P, n * NQ : (n + 1) * NQ
                                    ],
                                    in_=ot,
                                )
                            return t
                        thunks.append(mk(ms, n))
                return thunks

            def wo_tail_waves(c):
                """Final-chunk wo, kt-split: kt=0 (pair-0 atT, ready early)
                can issue during pair-1's norm chain; 4 concurrent PSUM
                groups (2 proj + 2 borrowed scores banks).  Output casts
                alternate Scalar/Vector, output DMA issues alternate
                Sync/GpSimd, so the tail drains ~2x faster."""
                units = [(4 * c + mb, n) for mb in range(4) for n in range(2)]
                waves = []
                for wave in (units[:4], units[4:]):
                    pss = {}
                    def mk_kt0(i, ms, n, pss):
                        def t():
                            if i < 2:
                                pss[i] = proj_ps.tile(
                                    [P, NQ], f32, name="proj", tag="proj"
                                )
                            else:
                                ps2 = scores_ps.tile(
                                    [P, 2, NQ], f32, name="sc2", tag="sc2"
                                )
                                pss[i] = ps2[:, 0, :]
                            nc.tensor.matmul(
                                pss[i],
                                atT[0][:, ms * P : (ms + 1) * P],
                                wo_sb[:, 0, n * NQ : (n + 1) * NQ],
                                start=True,
                                stop=False,
                            )
                        return t
                    def mk_kt1(i, ms, n, pss):
                        def t():
                            nc.tensor.matmul(
                                pss[i],
                                atT[1][:, ms * P : (ms + 1) * P],
                                wo_sb[:, 1, n * NQ : (n + 1) * NQ],
                                start=False,
                                stop=True,
                            )
                            ot = out_pool.tile([P, NQ], h16, name="ot", tag="ot")
                            if i % 2 == 0:
                                nc.vector.tensor_copy(ot, pss[i])
                            else:
                                nc.scalar.activation(
                                    out=ot, in_=pss[i], func=Identity, scale=1.0
                                )
                            eng = sy if i % 2 == 0 else nc.gpsimd
                            eng.dma_start(
                                out=out[ms * P : (ms + 1) * P, n * NQ : (n + 1) * NQ],
                                in_=ot,
                            )
                        return t
                    kt0s = [mk_kt0(i, ms, n, pss) for i, (ms, n) in enumerate(wave)]
                    kt1s = [mk_kt1(i, ms, n, pss) for i, (ms, n) in enumerate(wave)]
                    waves.append((kt0s, kt1s))
                return waves

            def emit_interleaved(primary, filler):
                fi = 0
                n = max(len(primary), 1)
                f = len(filler)
                for i, t in enumerate(primary):
                    t()
                    while fi * n < f * (i + 1):
                        filler[fi]()
                        fi += 1
                for t in filler[fi:]:
                    t()

            # --- main schedule ----------------------------------------------
            holder = {}
            # prologue: K -> Q -> V (scores need q+k first; PV can lag), then
            # attention consts, then chunk-1 x so round-0 filler is never
            # DMA-starved.  Emission order == sync-queue issue order ==
            # transfer priority; wk/xk pieces interleave so the first
            # projection matmul starts one small DMA piece into the stream.
            xk0 = load_x_thunk(xkt, 0, holder, ("xk", 0), parts=3)
            sc0, pv0, epi0 = attn_thunks(0, split=True)
            v0 = proj_v_thunks(0, holder, ("xv", 0))
            # attention chunk 0 rides inside the V projection: each j-block's
            # scores+exp+PV run right after vh[j] is built, so the Scalar
            # engine's exp stream starts during the V window (it otherwise
            # idles until every projection is done, then becomes the
            # bottleneck in the late rounds)
            v_att = []
            for mb in range(4):
                v_att += v0[9 * mb : 9 * (mb + 1)]
                v_att.append(sc0[0][mb])
                v_att.append(pv0[0][mb])
            v_att += epi0[0]
            for j in range(4):
                v_att.append(sc0[1][j])
                v_att.append(pv0[1][j])
            v_att += epi0[1]
            prologue = (
                [t_wk_p1, xk0[0], t_wk_p2, xk0[1], xk0[2]]
                + proj_qk_thunks(0, holder, ("xk", 0), wk_sb, bk_sb, khT)
                + [t_wq]
                + load_x_thunk(xqt, 0, holder, ("xq", 0))
                + proj_qk_thunks(0, holder, ("xq", 0), wq_sb, bq_sb, qhT)
                + [t_wv, t_attn_consts]
                + load_x_thunk(xvt, 0, holder, ("xv", 0))
                + v_att
                + load_x_thunk(xkt, 1, holder, ("xk", 1))
                + load_x_thunk(xqt, 1, holder, ("xq", 1))
                + load_x_thunk(xvt, 1, holder, ("xv", 1))
            )
            for t in prologue:
                t()
            kv_deferred = {}
            tail = None
            for c in range(QC):
                kv_filler = kv_deferred.pop(c, [])
                filler = []
                holdback = []
                if c == 0:
                    filler += [t_wo]
                if c == 3:
                    # round 3 is ACT-bound (exp over the longest k-ranges), so
                    # it absorbs the wo matmuls of chunks 0 and 1 as filler;
                    # all of wo(2) is held back to keep the PE warm through
                    # the final softmax epilogue chain
                    filler += wo_thunks(0) + wo_thunks(1)
                    holdback = wo_thunks(2)
                if c + 1 < QC:
                    if c + 2 < QC:
                        filler += load_x_thunk(xkt, c + 2, holder, ("xk", c + 2))
                        filler += load_x_thunk(xqt, c + 2, holder, ("xq", c + 2))
                        filler += load_x_thunk(xvt, c + 2, holder, ("xv", c + 2))
                    filler += proj_qk_thunks(
                        c + 1, holder, ("xq", c + 1), wq_sb, bq_sb, qhT
                    )
                    filler += proj_qk_thunks(
                        c + 1, holder, ("xk", c + 1), wk_sb, bk_sb, khT
                    )
                    kv_deferred[c + 1] = proj_v_thunks(
                        c + 1, holder, ("xv", c + 1)
                    )
                if c == 0:
                    for t in filler:
                        t()
                    continue
                att, tail = attn_thunks(c)
                seg1, seg2 = att[: 8 * c], att[8 * c :]
                emit_interleaved(seg1, kv_filler)
                # hold back a quarter of the filler to keep PE fed through the
                # end-of-round normalization chains
                cut = (3 * len(filler)) // 4
                emit_interleaved(seg2[:-8], filler[:cut])
                emit_interleaved(seg2[-8:], filler[cut:])
            # chunks whose wo was not emitted as filler above (QC=4 covers
            # chunks 0..2 in rounds 2/3; smaller QC emits them all here)
            done = {0, 1, 2} if QC == 4 else set()
            for cc in range(QC):
                if cc not in done and cc != QC - 1:
                    for t in wo_thunks(cc):
                        t()
                    holdback = []
            # ---- tail: last head-pair epilogue with independent wo work
            # interleaved so the PE queue never waits on the ACT chain ------
            (w1_kt0, w1_kt1), (w2_kt0, w2_kt1) = wo_tail_waves(QC - 1)
            if tail:
                for t in tail["act"]:       # ln0, exp0, ln1, exp1 (ACT)
                    t()
                for t in holdback:          # held-back wo(2) units (PE)
                    t()
                for t in w1_kt0:            # wave-1 kt0 (PE, needs atT[0])
                    t()
                for t in tail["bcast"]:     # PE broadcasts of 1/denom
                    t()
                for t in tail["mul"]:       # norm multiplies (DVE)
                    t()
            else:
                for t in holdback:
                    t()
                for t in w1_kt0:
                    t()
            for t in w1_kt1 + w2_kt0 + w2_kt1:
                t()

    nc.compile()
    return nc


def _get_nc(nc_s=S):
    if nc_s not in _CACHE:
        _CACHE[nc_s] = _build(nc_s)
    return _CACHE[nc_s]


def make_masks():
    m = np.zeros((P, 4, NQ), np.float32)
    ql = np.arange(NQ)[None, :]
    kl = np.arange(P)[:, None]
    for jj in range(4):
        m[:, jj, :] = (ql >= kl + jj * P).astype(np.float32)
    return m.reshape(P, 4 * NQ)


def _x_layout(x):
    """[S, E] activations -> [P, QC, KT, NQ]: per-partition-contiguous chunks
    so every x DMA is 128 x (up to 8KB) contiguous runs at full HBM rate."""
    QC = S // NQ
    xt = np.ascontiguousarray(x.T).astype(np.float16)          # [E, S]
    return np.ascontiguousarray(
        xt.reshape(KT, P, QC, NQ).transpose(1, 2, 0, 3)
    )


def _w_layout(w):
    # [E, CW] -> [P, KT, CW]
    return np.ascontiguousarray(
        w.reshape(KT, P, CW).transpose(1, 0, 2)
    ).astype(np.float16)


def _wo_layout(w):
    # [CW, E] -> [P, CW//P, E]
    return np.ascontiguousarray(
        w.reshape(CW // P, P, E).transpose(1, 0, 2)
    ).astype(np.float16)


def _b_layout(b):
    # [CW] -> [P, 2]
    return np.ascontiguousarray(b.reshape(2, P).T).astype(np.float32)


def make_in_maps(q, k, v, Wq, bq, Wk, bk, Wv, bv, Wo):
    masks = make_masks()
    in_maps = []
    for core in range(8):
        b, g = divmod(core, 4)
        cs = slice(g * CW, (g + 1) * CW)
        in_maps.append(
            {
                "xqt": _x_layout(q[b]),
                "xkt": _x_layout(k[b]),
                "xvt": _x_layout(v[b]),
                "wq": _w_layout(Wq[:, cs]),
                "wk": _w_layout(Wk[:, cs]),
                "wv": _w_layout(Wv[:, cs]),
                "wo": _wo_layout(Wo[cs, :]),
                "bq": _b_layout(bq[cs]),
                "bk": _b_layout(bk[cs]),
                "bv": np.ascontiguousarray(bv[cs].reshape(1, CW)).astype(np.float16),
                "masks": masks.astype(np.float16),
                "ones": np.ones((P, P), np.float16),
            }
        )
    return in_maps


def run(q, k, v, Wq, bq, Wk, bk, Wv, bv, Wo, bo, **run_kwargs):
    """Returns (output, BassKernelResults)."""
    from concourse.bass_utils import run_bass_kernel_spmd

    q, k, v = (np.asarray(x, np.float32) for x in (q, k, v))
    nc = _get_nc()
    in_maps = make_in_maps(
        q, k, v,
        np.asarray(Wq, np.float32), np.asarray(bq, np.float32),
        np.asarray(Wk, np.float32), np.asarray(bk, np.float32),
        np.asarray(Wv, np.float32), np.asarray(bv, np.float32),
        np.asarray(Wo, np.float32),
    )
    res = run_bass_kernel_spmd(nc, in_maps, list(range(8)), **run_kwargs)
    out = np.zeros((B, S, E), np.float32)
    for core in range(8):
        out[core // 4] += res.results[core]["out"].astype(np.float32)
    out += np.asarray(bo, np.float32)[None, None, :]
    return out, res


def kernel(q, k, v, Wq, bq, Wk, bk, Wv, bv, Wo, bo):
    return run(q, k, v, Wq, bq, Wk, bk, Wv, bv, Wo, bo)[0]
